# revision 4
# baseline (speedup 1.0000x reference)
"""Trainium2 Bass kernel for nn_LowRankSTLayer_dilation.

Mathematical reduction (validated vs the jax reference, ~6e-7 absmax rel):
  1. U/V start rank-symmetric and the multiplicative NMF updates preserve
     that, so the rank-3 iteration is exactly rank-1.
  2. eps=1e-6 is negligible vs the O(1)+ denominators, so each update is a
     plain normalized projection -- power iteration on the per-position
     Gram matrix G = X X^T.  All normalization scalars cancel:
         out = relu( tail_w @ ( p3 * <h,p2>/<p3,p2> ) )
     p0 = box27(h), p_{n+1} = G p_n, G = box27(h h^T) (separable 3x3x3
     box filter of the 136 channel-pair products), h = relu(head_w @ x).

Sharding: 8 cores = batch(2) x H-quarters(4); each core receives a
replicate-padded slice [17, 10, 26, 98] covering all 8 frames (+temporal
halo) for 24 output rows (channel 16 is constant 1.0, used to synthesize
constant rows via PE matmuls).  This amortizes the head conv / pair
products over all frames (1.36 padded positions per output vs 2.55 for
frame-pair sharding).

bf16 pipeline (PSUM accumulation fp32).  m2 is a 128-row tile carrying h
at rows 0-15 with replicas at 32-47/64-79 (so the power iteration's p
and the final gamma stage live at partition strips 0/32/64 and their
matmuls run concurrently in distinct PE tiles), diag-hi squares at
96-103 and duplicate pair products #96-119 at 104-127 (so the a-role
mul reads one tile at a legal partition base).
"""

import numpy as np
import ml_dtypes
from contextlib import ExitStack

import concourse.bass as bass
import concourse.bacc as bacc
import concourse.tile as tile
from concourse import mybir
from concourse.bass_utils import run_bass_kernel_spmd

F32 = mybir.dt.float32
BF16 = mybir.dt.bfloat16

B, C, D, H, W = 2, 16, 8, 96, 96
NCORES = 8
HP, WP = H + 2, W + 2            # spatially padded
HR = 24                           # output rows per core
DF = D + 2                        # frames incl. temporal halo (10)
R = 6                             # output rows per chunk
NCHUNK = HR // R                  # 4
RIN = R + 2                       # input rows per chunk (8)
PIN_F = RIN * WP                  # input positions per frame per chunk (784)
PIECE = PIN_F // 2                # matmul piece (392 <= 512)
POS = D * R * W                   # output positions per chunk (4608)
CPIECE = 512
NPC = POS // CPIECE               # 9 pieces, strips cycle 0/32/64
FR = R * W                        # 576 positions per frame per chunk
NPAIR = 120

_pairs = [(a, b) for a in range(C) for b in range(a + 1, C)]
_A = np.array([p[0] for p in _pairs])
_B = np.array([p[1] for p in _pairs])


def _build_consts(head_w, tail_w):
    hwT = head_w.T.astype(np.float32)          # [c_in, c_out]
    # head conv + ones passthrough: h16ext = relu([head_w @ x ; x_ones])
    w_head = np.zeros((C + 1, C + 1), np.float32)
    w_head[:C, :C] = hwT
    w_head[C, C] = 1.0
    # m1 rows: 0..119 pair products, 120..127 diag-lo squares (c=0..7)
    w_a = np.zeros((C + 1, 128), np.float32)
    w_b = np.zeros((C + 1, 128), np.float32)
    w_a[_A, np.arange(NPAIR)] = 1.0
    w_b[_B, np.arange(NPAIR)] = 1.0
    w_a[np.arange(8), NPAIR + np.arange(8)] = 1.0
    w_b[np.arange(8), NPAIR + np.arange(8)] = 1.0
    # m2 rows: h at 0..15 with replicas at 32..47 / 64..79, diag-hi
    # squares at 96..103, duplicate pairs #96..119 at 104..127
    w_f = np.zeros((C + 1, 128), np.float32)
    w_g = np.zeros((C + 1, 128), np.float32)
    for s in (0, 32, 64):
        w_f[np.arange(C), s + np.arange(16)] = 1.0
        w_g[C, s:s + 16] = 1.0
    w_f[8 + np.arange(8), 96 + np.arange(8)] = 1.0
    w_g[8 + np.arange(8), 96 + np.arange(8)] = 1.0
    w_f[_A[96:], 104 + np.arange(24)] = 1.0
    w_g[_B[96:], 104 + np.arange(24)] = 1.0
    # power-iter broadcasts: pia rows <- p[b] / p[c<8]; pibh rows <- p[a] / p[c>=8]
    sel_b = np.zeros((C, 128), np.float32)
    sel_b[_B, np.arange(NPAIR)] = 1.0
    sel_b[np.arange(8), NPAIR + np.arange(8)] = 1.0
    sel_ah = np.zeros((C, 128), np.float32)
    sel_ah[_A[:96], np.arange(96)] = 1.0
    sel_ah[8 + np.arange(8), 96 + np.arange(8)] = 1.0
    sel_ah[_A[96:], 104 + np.arange(24)] = 1.0
    # reductions: pia -> channel a / c<8 ; pibh -> channel b / c>=8
    s_a = np.zeros((128, C), np.float32)
    s_a[np.arange(NPAIR), _A] = 1.0
    s_a[NPAIR + np.arange(8), np.arange(8)] = 1.0
    s_bh = np.zeros((128, C), np.float32)
    s_bh[np.arange(96), _B[:96]] = 1.0
    s_bh[96 + np.arange(8), 8 + np.arange(8)] = 1.0
    s_bh[104 + np.arange(24), _B[96:]] = 1.0
    tail_t = tail_w.T.astype(np.float32).copy()
    return dict(w_head=w_head, w_a=w_a, w_b=w_b, w_f=w_f, w_g=w_g,
                sel_b=sel_b, sel_ah=sel_ah, s_a=s_a, s_bh=s_bh,
                tail_t=tail_t)


def _build_strip_consts(head_w, tail_w):
    """Stationary tensors replicated at partition strips so independent
    matmuls run concurrently in distinct 32-row/32-col PE tiles."""
    c = _build_consts(head_w, tail_w)
    out = {}
    # head conv: lhsT rows at 0 (rhs = x), output cols at strip 32k
    w_head_s = np.zeros((C + 1, 81), np.float32)
    for s in (0, 32):
        w_head_s[:, s:s + C + 1] = c["w_head"]
    out["w_head_s"] = w_head_s
    # pair-product selectors: contraction rows at strip 32k
    for k in ("w_a", "w_b", "w_f", "w_g"):
        m = np.zeros((49, 128), np.float32)
        for s in (0, 32):
            m[s:s + C + 1, :] = c[k]
        out[k + "_s"] = m
    # power-iteration broadcasts: contraction rows (p) at strip 32k
    for k in ("sel_b", "sel_ah"):
        m = np.zeros((80, 128), np.float32)
        for s in (0, 32, 64):
            m[s:s + C, :] = c[k]
        out[k + "_s"] = m
    # reductions: output cols (p channels) at strip 32k
    for k in ("s_a", "s_bh"):
        m = np.zeros((128, 80), np.float32)
        for s in (0, 32, 64):
            m[:, s:s + C] = c[k]
        out[k + "_s"] = m
    out["ones_a_s"] = np.ones((80, 1), np.float32)
    out["ones_g_s"] = np.ones((1, 80), np.float32)
    tail_s = np.zeros((80, 80), np.float32)
    for s in (0, 32, 64):
        tail_s[s:s + C, s:s + C] = c["tail_t"]
    out["tail_s"] = tail_s
    return out


_CONST_SHAPES = dict(w_head_s=(C + 1, 81), w_a_s=(49, 128),
                     w_b_s=(49, 128), w_f_s=(49, 128),
                     w_g_s=(49, 128), sel_b_s=(80, 128),
                     sel_ah_s=(80, 128),
                     s_a_s=(128, 80), s_bh_s=(128, 80),
                     ones_a_s=(80, 1), ones_g_s=(1, 80),
                     tail_s=(80, 80))


def _spans(pc):
    """Decompose flat piece [512*pc, 512*(pc+1)) into per-frame
    contiguous (frame, offset, len) spans (FR=576 positions/frame)."""
    q, end, out = CPIECE * pc, CPIECE * (pc + 1), []
    while q < end:
        f, qf = divmod(q, FR)
        ln = min(FR - qf, end - q)
        out.append((f, qf, ln))
        q += ln
    return out


def _build_program():
    nc = bacc.Bacc("TRN2", target_bir_lowering=False, debug=False)
    xin = nc.declare_dram_parameter("xin", [C + 1, DF, HR + 2, WP], BF16,
                                    isOutput=False)
    cst = {k: nc.declare_dram_parameter(k, list(v), BF16, isOutput=False)
           for k, v in _CONST_SHAPES.items()}
    out = nc.declare_dram_parameter("out", [C, D, HR, W], F32, isOutput=True)

    with tile.TileContext(nc) as tc, ExitStack() as ctx:
        singles = ctx.enter_context(tc.tile_pool(name="singles", bufs=1))
        sb = {}
        for k, v in _CONST_SHAPES.items():
            sb[k] = singles.tile(list(v), BF16, tag=k, name=k)
            nc.sync.dma_start(out=sb[k], in_=cst[k][:, :])

        xpool = ctx.enter_context(tc.tile_pool(name="x", bufs=2))
        ps = ctx.enter_context(tc.tile_pool(name="ps", bufs=2, space="PSUM"))
        mpool = ctx.enter_context(tc.tile_pool(name="m", bufs=1))
        boxp = ctx.enter_context(tc.tile_pool(name="box", bufs=1))
        gap = ctx.enter_context(tc.tile_pool(name="gap", bufs=1))
        pp = ctx.enter_context(tc.tile_pool(name="pp", bufs=1))
        outp = ctx.enter_context(tc.tile_pool(name="outp", bufs=2))
        gmp = ctx.enter_context(tc.tile_pool(name="gmp", bufs=2))

        def issue_stage1(ci):
            """DMA + head conv + pair products for chunk ci -> (m1, m2).

            The two 392-column pieces of each frame run in PE strips
            0/32: the head conv is column-tiled and the four
            pair-product matmuls are row-tiled, so pieces execute
            concurrently."""
            r0 = ci * R
            m1 = mpool.tile([128, DF, PIN_F], BF16, tag="m1")
            m2 = mpool.tile([128, DF, PIN_F], BF16, tag="m2")
            for f in range(DF):
                xs = xpool.tile([C + 1, RIN, WP], BF16)
                nc.sync.dma_start(out=xs, in_=xin[:, f, r0:r0 + RIN, :])
                xf = xs.rearrange("c r w -> c (r w)")
                hx = xpool.tile([49, PIECE], BF16, tag="hx")
                for pc in range(2):
                    s = 32 * pc
                    sl = slice(pc * PIECE, (pc + 1) * PIECE)
                    ph = ps.tile([49, PIECE], F32, tag="q0")
                    nc.tensor.matmul(ph[s:s + C + 1, :],
                                     sb["w_head_s"][:, s:s + C + 1],
                                     xf[:, sl], start=True, stop=True)
                    nc.scalar.activation(hx[s:s + C + 1, :],
                                         ph[s:s + C + 1, :],
                                         mybir.ActivationFunctionType.Relu)
                for pc in range(2):
                    s = 32 * pc
                    sl = slice(pc * PIECE, (pc + 1) * PIECE)
                    hxs = hx[s:s + C + 1, :]
                    pa = ps.tile([128, PIECE], F32, tag="q1")
                    nc.tensor.matmul(pa, sb["w_a_s"][s:s + C + 1, :], hxs,
                                     start=True, stop=True)
                    ha = xpool.tile([128, PIECE], BF16, tag="ha")
                    nc.scalar.copy(ha, pa)
                    pb = ps.tile([128, PIECE], F32, tag="q2")
                    nc.tensor.matmul(pb, sb["w_b_s"][s:s + C + 1, :], hxs,
                                     start=True, stop=True)
                    nc.vector.tensor_mul(m1[:, f, sl], ha, pb)
                    pf = ps.tile([128, PIECE], F32, tag="q3")
                    nc.tensor.matmul(pf, sb["w_f_s"][s:s + C + 1, :], hxs,
                                     start=True, stop=True)
                    hf = xpool.tile([128, PIECE], BF16, tag="hf")
                    nc.scalar.copy(hf, pf)
                    pg = ps.tile([128, PIECE], F32, tag="q0")
                    nc.tensor.matmul(pg, sb["w_g_s"][s:s + C + 1, :], hxs,
                                     start=True, stop=True)
                    nc.vector.tensor_mul(m2[:, f, sl], hf, pg)
            return m1, m2

        def box_di(src, tagp, eng_d, eng_i):
            v = src.rearrange("p f (r w) -> p f r w", w=WP)
            t0 = boxp.tile([128, D, RIN, WP], BF16, tag=f"tmp{tagp}")
            bd = boxp.tile([128, D, RIN, WP], BF16, tag=f"bd{tagp}")
            eng_d.tensor_add(t0, v[:, 0:D], v[:, 1:D + 1])
            eng_d.tensor_add(bd, t0, v[:, 2:D + 2])
            t1 = boxp.tile([128, D, R, WP], BF16, tag=f"tmp{tagp}")
            bi = boxp.tile([128, D, R, WP], BF16, tag=f"bi{tagp}")
            eng_i.tensor_add(t1, bd[:, :, 0:R], bd[:, :, 1:R + 1])
            eng_i.tensor_add(bi, t1, bd[:, :, 2:R + 2])
            return bi

        def box_j(bi, tagp, eng_j):
            t2 = boxp.tile([128, D, R, W], BF16, tag=f"tmp{tagp}")
            bj = boxp.tile([128, D, R, W], BF16, tag=f"bd{tagp}")
            eng_j.tensor_add(t2, bi[:, :, :, 0:W], bi[:, :, :, 2:W + 2])
            eng_j.tensor_add(bj, t2, bi[:, :, :, 1:W + 1])
            return bj

        def issue_power_final(ci, g1, g2, m2):
            """Power iteration + gamma/tail for chunk ci.

            The nine 512-position pieces run in PE strips 0/32/64 (pc
            mod 3): the p iterate for piece pc lives at partitions
            32*(pc%3) (p0 comes from the h replicas), so broadcast
            matmuls are row-tiled and reduction/final matmuls are
            column-tiled."""
            g1v = g1.rearrange("p f r w -> p (f r w)")
            g2v = g2.rearrange("p f r w -> p (f r w)")
            # ---- power iteration: p_{n+1} = G p_n ----
            p_bufs = []
            p_cur = g2v
            for app in range(3):
                pia = gap.tile([128, POS], BF16, tag="pia")
                pibh = gap.tile([128, POS], BF16, tag="pibh")
                pnx = pp.tile([80, POS], BF16,
                              tag="pA" if app != 1 else "pB")
                for pc in range(NPC):
                    s = 32 * (pc % 3)
                    sl = slice(pc * CPIECE, (pc + 1) * CPIECE)
                    pcs = p_cur[s:s + C, sl]
                    prb = ps.tile([128, CPIECE], F32, tag="q0")
                    prab = ps.tile([128, CPIECE], F32, tag="q1")
                    nc.tensor.matmul(prb, sb["sel_b_s"][s:s + C, :], pcs,
                                     start=True, stop=True)
                    nc.tensor.matmul(prab, sb["sel_ah_s"][s:s + C, :], pcs,
                                     start=True, stop=True)
                    nc.vector.tensor_mul(pia[:, sl], g1v[:, sl], prb)
                    nc.vector.tensor_mul(pibh[0:96, sl],
                                         g1v[0:96, sl], prab[0:96, :])
                    nc.vector.tensor_mul(pibh[96:128, sl],
                                         g2v[96:128, sl], prab[96:128, :])
                    acc = ps.tile([80, CPIECE], F32, tag="q2")
                    nc.tensor.matmul(acc[s:s + C, :],
                                     sb["s_a_s"][:, s:s + C], pia[:, sl],
                                     start=True, stop=False)
                    nc.tensor.matmul(acc[s:s + C, :],
                                     sb["s_bh_s"][:, s:s + C], pibh[:, sl],
                                     start=False, stop=True)
                    nc.scalar.copy(pnx[s:s + C, sl], acc[s:s + C, :])
                p_bufs.append(pnx)
                p_cur = pnx
            p2, p3 = p_bufs[1], p_bufs[2]

            # ---- gamma = <h,p2>/<p3,p2>; out = relu(tail (gamma*p3)) ----
            # contiguous copy of the centre h values, one per strip
            hcr = gap.tile([80, D, R, W], BF16, tag="hcr")
            for k in range(3):
                s = 32 * k
                nc.scalar.copy(
                    hcr[s:s + C], m2[s:s + C, 1:D + 1, :].rearrange(
                        "c f (r w) -> c f r w", w=WP)[:, :, 1:R + 1,
                                                      1:W + 1])
            hcrv = hcr.rearrange("c f r w -> c (f r w)")
            thn = gap.tile([80, POS], BF16, tag="thn")
            tdn = gap.tile([80, POS], BF16, tag="tdn")
            ofv = out.rearrange("c f h w -> c f (h w)")
            for pc in range(NPC):
                s = 32 * (pc % 3)
                sl = slice(pc * CPIECE, (pc + 1) * CPIECE)
                nc.vector.tensor_mul(thn[s:s + C, sl], hcrv[s:s + C, sl],
                                     p2[s:s + C, sl])
                nc.vector.tensor_mul(tdn[s:s + C, sl], p3[s:s + C, sl],
                                     p2[s:s + C, sl])
                pnum = ps.tile([1, CPIECE], F32, tag="q3")
                pden = ps.tile([1, CPIECE], F32, tag="q0")
                nc.tensor.matmul(pnum, sb["ones_a_s"][s:s + C, :],
                                 thn[s:s + C, sl], start=True, stop=True)
                nc.tensor.matmul(pden, sb["ones_a_s"][s:s + C, :],
                                 tdn[s:s + C, sl], start=True, stop=True)
                gam = gmp.tile([1, CPIECE], BF16, tag="gam")
                rcp = gmp.tile([1, CPIECE], F32, tag="rcp")
                nc.vector.reciprocal_approx_fast(out=rcp, in_=pden)
                nc.vector.tensor_mul(gam, pnum, rcp)
                grep = ps.tile([80, CPIECE], F32, tag="q1")
                nc.tensor.matmul(grep[s:s + C, :],
                                 sb["ones_g_s"][:, s:s + C],
                                 gam, start=True, stop=True)
                upre = gap.tile([80, CPIECE], BF16, tag="upre", bufs=2)
                nc.vector.tensor_mul(upre[s:s + C, :], p3[s:s + C, sl],
                                     grep[s:s + C, :])
                pout = ps.tile([80, CPIECE], F32, tag="q2")
                nc.tensor.matmul(pout[s:s + C, :],
                                 sb["tail_s"][s:s + C, s:s + C],
                                 upre[s:s + C, :], start=True, stop=True)
                osb = outp.tile([80, CPIECE], F32, tag="osb")
                nc.scalar.activation(osb[s:s + C, :], pout[s:s + C, :],
                                     mybir.ActivationFunctionType.Relu)
                for f, q0, ln in _spans(pc):
                    nc.sync.dma_start(
                        out=ofv[:, f, ci * FR + q0:ci * FR + q0 + ln],
                        in_=osb[s:s + C,
                                f * FR + q0 - pc * CPIECE:
                                f * FR + q0 - pc * CPIECE + ln])

        # Software-pipelined chunk loop: stage1 of chunk ci+1 is issued
        # between box(ci) and power(ci) so the PE always has matmul work
        # while vector/gpsimd run the box filter, and gpsimd's m2 box
        # overlaps the vector muls of chunk ci+1.
        st = issue_stage1(0)
        for ci in range(NCHUNK):
            m1, m2 = st
            bi1 = box_di(m1, "1", nc.vector, nc.vector)
            g1 = box_j(bi1, "1", nc.vector)
            bi2 = box_di(m2, "2", nc.gpsimd, nc.gpsimd)
            g2 = box_j(bi2, "2", nc.gpsimd)
            if ci + 1 < NCHUNK:
                st = issue_stage1(ci + 1)
            issue_power_final(ci, g1, g2, m2)
    nc.compile()
    return nc


_NC_CACHE = None
TRACE = False
LAST_EXEC_NS = None
LAST_RESULT = None


def kernel(x, head_w, tail_w):
    global _NC_CACHE, LAST_EXEC_NS, LAST_RESULT
    x = np.asarray(x, dtype=np.float32)
    head_w = np.asarray(head_w, dtype=np.float32)
    tail_w = np.asarray(tail_w, dtype=np.float32)

    consts = {k: v.astype(ml_dtypes.bfloat16)
              for k, v in _build_strip_consts(head_w, tail_w).items()}
    xp = np.pad(x, ((0, 0), (0, 0), (1, 1), (1, 1), (1, 1)), mode="edge")
    in_maps = []
    for core in range(NCORES):
        b, hs = divmod(core, 4)
        xs = np.empty((C + 1, DF, HR + 2, WP), ml_dtypes.bfloat16)
        xs[:C] = xp[b, :, :, HR * hs:HR * hs + HR + 2, :]
        xs[C] = 1.0
        m = {"xin": xs}
        m.update(consts)
        in_maps.append(m)

    if _NC_CACHE is None:
        _NC_CACHE = _build_program()
    res = run_bass_kernel_spmd(_NC_CACHE, in_maps, list(range(NCORES)),
                               trace=TRACE)
    LAST_EXEC_NS = res.exec_time_ns
    LAST_RESULT = res

    outf = np.empty((B, C, D, H, W), np.float32)
    for core in range(NCORES):
        b, hs = divmod(core, 4)
        outf[b, :, :, HR * hs:HR * hs + HR] = res.results[core]["out"]
    return outf



# revision 7
# speedup vs baseline: 1.1901x; 1.1901x over previous
"""Trainium2 Bass kernel for nn_LowRankSTLayer_dilation.

Mathematical reduction (validated vs the jax reference, ~6e-7 absmax rel):
  1. U/V start rank-symmetric and the multiplicative NMF updates preserve
     that, so the rank-3 iteration is exactly rank-1.
  2. eps=1e-6 is negligible vs the O(1)+ denominators, so each update is a
     plain normalized projection -- power iteration on the per-position
     Gram matrix G = X X^T.  All normalization scalars cancel:
         out = relu( tail_w @ ( p3 * <h,p2>/<p3,p2> ) )
     p0 = box27(h), p_{n+1} = G p_n, G = box27(h h^T) (separable 3x3x3
     box filter of the 136 channel-pair products), h = relu(head_w @ x).

Sharding: 8 cores = batch(2) x H-quarters(4); each core receives a
replicate-padded slice [17, 10, 26, 98] covering all 8 frames (+temporal
halo) for 24 output rows (channel 16 is constant 1.0, used to synthesize
constant rows via PE matmuls).  This amortizes the head conv / pair
products over all frames (1.36 padded positions per output vs 2.55 for
frame-pair sharding).

bf16 pipeline (PSUM accumulation fp32).  m2 is a 128-row tile carrying h
at rows 0-15 with replicas at 32-47/64-79 (so the power iteration's p
and the final gamma stage live at partition strips 0/32/64 and their
matmuls run concurrently in distinct PE tiles), diag-hi squares at
96-103 and duplicate pair products #96-119 at 104-127 (so the a-role
mul reads one tile at a legal partition base).
"""

import numpy as np
import ml_dtypes
from contextlib import ExitStack

import concourse.bass as bass
import concourse.bacc as bacc
import concourse.tile as tile
from concourse import mybir
from concourse.bass_utils import run_bass_kernel_spmd

F32 = mybir.dt.float32
BF16 = mybir.dt.bfloat16

B, C, D, H, W = 2, 16, 8, 96, 96
NCORES = 8
HP, WP = H + 2, W + 2            # spatially padded
HR = 24                           # output rows per core
DF = D + 2                        # frames incl. temporal halo (10)
R = 6                             # output rows per chunk
NCHUNK = HR // R                  # 4
RIN = R + 2                       # input rows per chunk (8)
PIN_F = RIN * WP                  # input positions per frame per chunk (784)
PIECE = PIN_F // 2                # matmul piece (392 <= 512)
POS = D * R * W                   # output positions per chunk (4608)
CPIECE = 512
NPC = POS // CPIECE               # 9 pieces, strips cycle 0/32/64
FR = R * W                        # 576 positions per frame per chunk
NPAIR = 120

_pairs = [(a, b) for a in range(C) for b in range(a + 1, C)]
_A = np.array([p[0] for p in _pairs])
_B = np.array([p[1] for p in _pairs])


def _build_consts(head_w, tail_w):
    hwT = head_w.T.astype(np.float32)          # [c_in, c_out]
    # head conv + ones passthrough: h16ext = relu([head_w @ x ; x_ones])
    w_head = np.zeros((C + 1, C + 1), np.float32)
    w_head[:C, :C] = hwT
    w_head[C, C] = 1.0
    # m1 rows: 0..119 pair products, 120..127 diag-lo squares (c=0..7)
    w_a = np.zeros((C + 1, 128), np.float32)
    w_b = np.zeros((C + 1, 128), np.float32)
    w_a[_A, np.arange(NPAIR)] = 1.0
    w_b[_B, np.arange(NPAIR)] = 1.0
    w_a[np.arange(8), NPAIR + np.arange(8)] = 1.0
    w_b[np.arange(8), NPAIR + np.arange(8)] = 1.0
    # m2 rows: h at 0..15 with replicas at 32..47 / 64..79, diag-hi
    # squares at 96..103, duplicate pairs #96..119 at 104..127
    w_f = np.zeros((C + 1, 128), np.float32)
    w_g = np.zeros((C + 1, 128), np.float32)
    for s in (0, 32, 64):
        w_f[np.arange(C), s + np.arange(16)] = 1.0
        w_g[C, s:s + 16] = 1.0
    w_f[8 + np.arange(8), 96 + np.arange(8)] = 1.0
    w_g[8 + np.arange(8), 96 + np.arange(8)] = 1.0
    w_f[_A[96:], 104 + np.arange(24)] = 1.0
    w_g[_B[96:], 104 + np.arange(24)] = 1.0
    # power-iter broadcasts: pia rows <- p[b] / p[c<8]; pibh rows <- p[a] / p[c>=8]
    sel_b = np.zeros((C, 128), np.float32)
    sel_b[_B, np.arange(NPAIR)] = 1.0
    sel_b[np.arange(8), NPAIR + np.arange(8)] = 1.0
    sel_ah = np.zeros((C, 128), np.float32)
    sel_ah[_A[:96], np.arange(96)] = 1.0
    sel_ah[8 + np.arange(8), 96 + np.arange(8)] = 1.0
    sel_ah[_A[96:], 104 + np.arange(24)] = 1.0
    # reductions: pia -> channel a / c<8 ; pibh -> channel b / c>=8
    s_a = np.zeros((128, C), np.float32)
    s_a[np.arange(NPAIR), _A] = 1.0
    s_a[NPAIR + np.arange(8), np.arange(8)] = 1.0
    s_bh = np.zeros((128, C), np.float32)
    s_bh[np.arange(96), _B[:96]] = 1.0
    s_bh[96 + np.arange(8), 8 + np.arange(8)] = 1.0
    s_bh[104 + np.arange(24), _B[96:]] = 1.0
    tail_t = tail_w.T.astype(np.float32).copy()
    return dict(w_head=w_head, w_a=w_a, w_b=w_b, w_f=w_f, w_g=w_g,
                sel_b=sel_b, sel_ah=sel_ah, s_a=s_a, s_bh=s_bh,
                tail_t=tail_t)


def _build_strip_consts(head_w, tail_w):
    """Stationary tensors replicated at partition strips so independent
    matmuls run concurrently in distinct 32-row/32-col PE tiles."""
    c = _build_consts(head_w, tail_w)
    out = {}
    # head conv: lhsT rows at 0 (rhs = x), output cols at strip 32k
    w_head_s = np.zeros((C + 1, 81), np.float32)
    for s in (0, 32):
        w_head_s[:, s:s + C + 1] = c["w_head"]
    out["w_head_s"] = w_head_s
    # pair-product selectors: contraction rows at strip 32k
    for k in ("w_a", "w_b", "w_f", "w_g"):
        m = np.zeros((49, 128), np.float32)
        for s in (0, 32):
            m[s:s + C + 1, :] = c[k]
        out[k + "_s"] = m
    # power-iteration broadcasts: contraction rows (p) at strip 32k
    for k in ("sel_b", "sel_ah"):
        m = np.zeros((80, 128), np.float32)
        for s in (0, 32, 64):
            m[s:s + C, :] = c[k]
        out[k + "_s"] = m
    # reductions: output cols (p channels) at strip 32k
    for k in ("s_a", "s_bh"):
        m = np.zeros((128, 80), np.float32)
        for s in (0, 32, 64):
            m[:, s:s + C] = c[k]
        out[k + "_s"] = m
    out["ones_a_s"] = np.ones((80, 1), np.float32)
    out["ones_g_s"] = np.ones((1, 80), np.float32)
    tail_s = np.zeros((80, 80), np.float32)
    for s in (0, 32, 64):
        tail_s[s:s + C, s:s + C] = c["tail_t"]
    out["tail_s"] = tail_s
    return out


_CONST_SHAPES = dict(w_head_s=(C + 1, 81), w_a_s=(49, 128),
                     w_b_s=(49, 128), w_f_s=(49, 128),
                     w_g_s=(49, 128), sel_b_s=(80, 128),
                     sel_ah_s=(80, 128),
                     s_a_s=(128, 80), s_bh_s=(128, 80),
                     ones_a_s=(80, 1), ones_g_s=(1, 80),
                     tail_s=(80, 80))


def _spans(pc):
    """Decompose flat piece [512*pc, 512*(pc+1)) into per-frame
    contiguous (frame, offset, len) spans (FR=576 positions/frame)."""
    q, end, out = CPIECE * pc, CPIECE * (pc + 1), []
    while q < end:
        f, qf = divmod(q, FR)
        ln = min(FR - qf, end - q)
        out.append((f, qf, ln))
        q += ln
    return out


def _build_program():
    nc = bacc.Bacc("TRN2", target_bir_lowering=False, debug=False)
    xin = nc.declare_dram_parameter("xin", [C + 1, DF, HR + 2, WP], BF16,
                                    isOutput=False)
    cst = {k: nc.declare_dram_parameter(k, list(v), BF16, isOutput=False)
           for k, v in _CONST_SHAPES.items()}
    out = nc.declare_dram_parameter("out", [C, D, HR, W], F32, isOutput=True)

    with tile.TileContext(nc) as tc, ExitStack() as ctx:
        singles = ctx.enter_context(tc.tile_pool(name="singles", bufs=1))
        sb = {}
        for k, v in _CONST_SHAPES.items():
            sb[k] = singles.tile(list(v), BF16, tag=k, name=k)
            nc.sync.dma_start(out=sb[k], in_=cst[k][:, :])

        xpool = ctx.enter_context(tc.tile_pool(name="x", bufs=2))
        ps = ctx.enter_context(tc.tile_pool(name="ps", bufs=2, space="PSUM"))
        mpool = ctx.enter_context(tc.tile_pool(name="m", bufs=1))
        boxp = ctx.enter_context(tc.tile_pool(name="box", bufs=1))
        gap = ctx.enter_context(tc.tile_pool(name="gap", bufs=1))
        pp = ctx.enter_context(tc.tile_pool(name="pp", bufs=1))
        outp = ctx.enter_context(tc.tile_pool(name="outp", bufs=2))
        gmp = ctx.enter_context(tc.tile_pool(name="gmp", bufs=2))

        def issue_stage1(ci):
            """DMA + head conv + pair products for chunk ci -> (m1, m2).

            The two 392-column pieces of each frame run in PE strips
            0/32: the head conv is column-tiled and the four
            pair-product matmuls are row-tiled, so pieces execute
            concurrently."""
            r0 = ci * R
            m1 = mpool.tile([128, DF, PIN_F], BF16, tag="m1")
            m2 = mpool.tile([128, DF, PIN_F], BF16, tag="m2")
            for f in range(DF):
                xs = xpool.tile([C + 1, RIN, WP], BF16)
                nc.sync.dma_start(out=xs, in_=xin[:, f, r0:r0 + RIN, :])
                xf = xs.rearrange("c r w -> c (r w)")
                hx = xpool.tile([49, PIECE], BF16, tag="hx")
                for pc in range(2):
                    s = 32 * pc
                    sl = slice(pc * PIECE, (pc + 1) * PIECE)
                    ph = ps.tile([49, PIECE], F32, tag="q0")
                    nc.tensor.matmul(ph[s:s + C + 1, :],
                                     sb["w_head_s"][:, s:s + C + 1],
                                     xf[:, sl], start=True, stop=True)
                    nc.scalar.activation(hx[s:s + C + 1, :],
                                         ph[s:s + C + 1, :],
                                         mybir.ActivationFunctionType.Relu)
                for pc in range(2):
                    s = 32 * pc
                    sl = slice(pc * PIECE, (pc + 1) * PIECE)
                    hxs = hx[s:s + C + 1, :]
                    pa = ps.tile([128, PIECE], F32, tag="q1")
                    nc.tensor.matmul(pa, sb["w_a_s"][s:s + C + 1, :], hxs,
                                     start=True, stop=True)
                    ha = xpool.tile([128, PIECE], BF16, tag="ha")
                    nc.scalar.copy(ha, pa)
                    pb = ps.tile([128, PIECE], F32, tag="q2")
                    nc.tensor.matmul(pb, sb["w_b_s"][s:s + C + 1, :], hxs,
                                     start=True, stop=True)
                    nc.vector.tensor_mul(m1[:, f, sl], ha, pb)
                    pf = ps.tile([128, PIECE], F32, tag="q3")
                    nc.tensor.matmul(pf, sb["w_f_s"][s:s + C + 1, :], hxs,
                                     start=True, stop=True)
                    hf = xpool.tile([128, PIECE], BF16, tag="hf")
                    nc.scalar.copy(hf, pf)
                    pg = ps.tile([128, PIECE], F32, tag="q0")
                    nc.tensor.matmul(pg, sb["w_g_s"][s:s + C + 1, :], hxs,
                                     start=True, stop=True)
                    nc.vector.tensor_mul(m2[:, f, sl], hf, pg)
            return m1, m2

        def box_di(src, tagp, eng_d, eng_i):
            v = src.rearrange("p f (r w) -> p f r w", w=WP)
            t0 = boxp.tile([128, D, RIN, WP], BF16, tag=f"tmp{tagp}")
            bd = boxp.tile([128, D, RIN, WP], BF16, tag=f"bd{tagp}")
            eng_d.tensor_add(t0, v[:, 0:D], v[:, 1:D + 1])
            eng_d.tensor_add(bd, t0, v[:, 2:D + 2])
            t1 = boxp.tile([128, D, R, WP], BF16, tag=f"tmp{tagp}")
            bi = boxp.tile([128, D, R, WP], BF16, tag=f"bi{tagp}")
            eng_i.tensor_add(t1, bd[:, :, 0:R], bd[:, :, 1:R + 1])
            eng_i.tensor_add(bi, t1, bd[:, :, 2:R + 2])
            return bi

        def box_j(bi, tagp, eng_j):
            t2 = boxp.tile([128, D, R, W], BF16, tag=f"tmp{tagp}")
            bj = boxp.tile([128, D, R, W], BF16, tag=f"bd{tagp}")
            eng_j.tensor_add(t2, bi[:, :, :, 0:W], bi[:, :, :, 2:W + 2])
            eng_j.tensor_add(bj, t2, bi[:, :, :, 1:W + 1])
            return bj

        def issue_power_final(ci, g1, g2, m2):
            """Power iteration + gamma/tail for chunk ci.

            The nine 512-position pieces run in PE strips 0/32/64 (pc
            mod 3): the p iterate for piece pc lives at partitions
            32*(pc%3) (p0 comes from the h replicas), so broadcast
            matmuls are row-tiled and reduction/final matmuls are
            column-tiled."""
            g1v = g1.rearrange("p f r w -> p (f r w)")
            g2v = g2.rearrange("p f r w -> p (f r w)")
            # packed source for the b-side mul: rows 0-95 from g1 (pairs
            # a-role), rows 96-127 from g2 (hi squares + dup pairs), so
            # the pibh product is one full-width DVE op instead of two.
            tb = gap.tile([128, POS], BF16, tag="tb")
            nc.vector.tensor_copy(out=tb[0:96, :], in_=g1v[0:96, :])
            nc.vector.tensor_copy(out=tb[96:128, :], in_=g2v[96:128, :])
            # ---- power iteration: p_{n+1} = G p_n ----
            p_bufs = []
            p_cur = g2v
            for app in range(3):
                pia = gap.tile([128, POS], BF16, tag="pia")
                pibh = gap.tile([128, POS], BF16, tag="pibh")
                pnx = pp.tile([80, POS], BF16,
                              tag="pA" if app != 1 else "pB")
                for pc in range(NPC):
                    s = 32 * (pc % 3)
                    sl = slice(pc * CPIECE, (pc + 1) * CPIECE)
                    pcs = p_cur[s:s + C, sl]
                    prb = ps.tile([128, CPIECE], F32, tag="q0")
                    prab = ps.tile([128, CPIECE], F32, tag="q1")
                    nc.tensor.matmul(prb, sb["sel_b_s"][s:s + C, :], pcs,
                                     start=True, stop=True)
                    nc.tensor.matmul(prab, sb["sel_ah_s"][s:s + C, :], pcs,
                                     start=True, stop=True)
                    nc.vector.tensor_mul(pia[:, sl], g1v[:, sl], prb)
                    nc.vector.tensor_mul(pibh[:, sl], tb[:, sl], prab)
                    acc = ps.tile([80, CPIECE], F32, tag="q2")
                    nc.tensor.matmul(acc[s:s + C, :],
                                     sb["s_a_s"][:, s:s + C], pia[:, sl],
                                     start=True, stop=False)
                    nc.tensor.matmul(acc[s:s + C, :],
                                     sb["s_bh_s"][:, s:s + C], pibh[:, sl],
                                     start=False, stop=True)
                    nc.scalar.copy(pnx[s:s + C, sl], acc[s:s + C, :])
                p_bufs.append(pnx)
                p_cur = pnx
            p2, p3 = p_bufs[1], p_bufs[2]

            # ---- gamma = <h,p2>/<p3,p2>; out = relu(tail (gamma*p3)) ----
            # contiguous copy of the centre h values, one per strip
            hcr = gap.tile([80, D, R, W], BF16, tag="hcr")
            for k in range(3):
                s = 32 * k
                nc.scalar.copy(
                    hcr[s:s + C], m2[s:s + C, 1:D + 1, :].rearrange(
                        "c f (r w) -> c f r w", w=WP)[:, :, 1:R + 1,
                                                      1:W + 1])
            hcrv = hcr.rearrange("c f r w -> c (f r w)")
            thn = gap.tile([80, POS], BF16, tag="thn")
            tdn = gap.tile([80, POS], BF16, tag="tdn")
            ofv = out.rearrange("c f h w -> c f (h w)")
            for pc in range(NPC):
                s = 32 * (pc % 3)
                sl = slice(pc * CPIECE, (pc + 1) * CPIECE)
                nc.vector.tensor_mul(thn[s:s + C, sl], hcrv[s:s + C, sl],
                                     p2[s:s + C, sl])
                nc.vector.tensor_mul(tdn[s:s + C, sl], p3[s:s + C, sl],
                                     p2[s:s + C, sl])
                pnum = ps.tile([1, CPIECE], F32, tag="q3")
                pden = ps.tile([1, CPIECE], F32, tag="q0")
                nc.tensor.matmul(pnum, sb["ones_a_s"][s:s + C, :],
                                 thn[s:s + C, sl], start=True, stop=True)
                nc.tensor.matmul(pden, sb["ones_a_s"][s:s + C, :],
                                 tdn[s:s + C, sl], start=True, stop=True)
                gam = gmp.tile([1, CPIECE], BF16, tag="gam")
                rcp = gmp.tile([1, CPIECE], F32, tag="rcp")
                nc.vector.reciprocal_approx_fast(out=rcp, in_=pden)
                nc.vector.tensor_mul(gam, pnum, rcp)
                grep = ps.tile([80, CPIECE], F32, tag="q1")
                nc.tensor.matmul(grep[s:s + C, :],
                                 sb["ones_g_s"][:, s:s + C],
                                 gam, start=True, stop=True)
                upre = gap.tile([80, CPIECE], BF16, tag="upre", bufs=2)
                nc.vector.tensor_mul(upre[s:s + C, :], p3[s:s + C, sl],
                                     grep[s:s + C, :])
                pout = ps.tile([80, CPIECE], F32, tag="q2")
                nc.tensor.matmul(pout[s:s + C, :],
                                 sb["tail_s"][s:s + C, s:s + C],
                                 upre[s:s + C, :], start=True, stop=True)
                osb = outp.tile([80, CPIECE], F32, tag="osb")
                nc.scalar.activation(osb[s:s + C, :], pout[s:s + C, :],
                                     mybir.ActivationFunctionType.Relu)
                for f, q0, ln in _spans(pc):
                    nc.sync.dma_start(
                        out=ofv[:, f, ci * FR + q0:ci * FR + q0 + ln],
                        in_=osb[s:s + C,
                                f * FR + q0 - pc * CPIECE:
                                f * FR + q0 - pc * CPIECE + ln])

        # Software-pipelined chunk loop: stage1 of chunk ci+1 is issued
        # between box(ci) and power(ci) so the PE always has matmul work
        # while vector/gpsimd run the box filter, and gpsimd's m2 box
        # overlaps the vector muls of chunk ci+1.
        st = issue_stage1(0)
        for ci in range(NCHUNK):
            m1, m2 = st
            bi1 = box_di(m1, "1", nc.vector, nc.vector)
            g1 = box_j(bi1, "1", nc.vector)
            bi2 = box_di(m2, "2", nc.gpsimd, nc.gpsimd)
            g2 = box_j(bi2, "2", nc.gpsimd)
            if ci + 1 < NCHUNK:
                st = issue_stage1(ci + 1)
            issue_power_final(ci, g1, g2, m2)
    nc.compile()
    return nc


_NC_CACHE = None
TRACE = False
LAST_EXEC_NS = None
LAST_RESULT = None


def kernel(x, head_w, tail_w):
    global _NC_CACHE, LAST_EXEC_NS, LAST_RESULT
    x = np.asarray(x, dtype=np.float32)
    head_w = np.asarray(head_w, dtype=np.float32)
    tail_w = np.asarray(tail_w, dtype=np.float32)

    consts = {k: v.astype(ml_dtypes.bfloat16)
              for k, v in _build_strip_consts(head_w, tail_w).items()}
    xp = np.pad(x, ((0, 0), (0, 0), (1, 1), (1, 1), (1, 1)), mode="edge")
    in_maps = []
    for core in range(NCORES):
        b, hs = divmod(core, 4)
        xs = np.empty((C + 1, DF, HR + 2, WP), ml_dtypes.bfloat16)
        xs[:C] = xp[b, :, :, HR * hs:HR * hs + HR + 2, :]
        xs[C] = 1.0
        m = {"xin": xs}
        m.update(consts)
        in_maps.append(m)

    if _NC_CACHE is None:
        _NC_CACHE = _build_program()
    res = run_bass_kernel_spmd(_NC_CACHE, in_maps, list(range(NCORES)),
                               trace=TRACE)
    LAST_EXEC_NS = res.exec_time_ns
    LAST_RESULT = res

    outf = np.empty((B, C, D, H, W), np.float32)
    for core in range(NCORES):
        b, hs = divmod(core, 4)
        outf[b, :, :, HR * hs:HR * hs + HR] = res.results[core]["out"]
    return outf

